# revision 3
# baseline (speedup 1.0000x reference)
"""Trainium2 Bass kernel for nn_CircuitRankNet (2-layer GCN siamese + mean-pool + MLP).

Algebraic collapse (no nonlinearity between the GCN layers): with
M = D^-1/2 (A+I) D^-1/2 and B = onehot(batch), the pooled embeddings need only
    P[g,d] = sum_over_aug_edges Chat[dst_e, g] * Xhat[src_e, d]
where Chat[i,:] = dinv_i^2 * sum_{e: src=i} dinv_dst * onehot64(batch[dst])
and Xhat[j,:] = dinv_j * X[j,:] (both computed on host from indices/degrees).

Device evaluation (v3, src-sharded):
- nodes split into 8 contiguous src-ranges (aug-outdegree balanced), one per
  NeuronCore; windows = fixed 128-node spans of the local range.
- per window, its incident edges (by src) are bucketed by residue
  r = dst % 4 and padded into exactly WB blocks of 128 edges per residue.
  Residue views chat[(25000,4,64)][:, r, :] keep gather indices (dst//4)
  int16-safe, with uniform density across windows.
- per residue, the window-major block stream is gathered from HBM in
  CHUNK-index dma_gather calls (8-way parallel Q7 descriptor generation)
  into an SBUF ring.
- per window: one-hot of the src slot (iota compare), per-block matmuls
  accumulate S_w[slot, g] in PSUM, then P += S_w^T @ Xhat[window rows]
  (second matmul, PSUM-accumulated across all windows).
- rare (window, residue) buckets over capacity spill to an exact host-side
  outer-product fixup; per-graph P partials are summed on the host and fed
  to the tiny compare MLP.
"""
import numpy as np

NCORES = 8
N = 100000
E = 1600000
G = 64
DIN = 128
NRES = 4
NQ = N // NRES        # 25000 rows per residue view
WB = 5                # blocks (of 128 edges) per residue per window
WCOLS = NRES * WB     # 20 one-hot columns per window
CAP = WB * 128        # 640-edge capacity per (window, residue)
CHUNK = 1024          # idxs per dma_gather call
CBLK = CHUNK // 128   # blocks per chunk
SCRATCH = 65536       # SWDGE descriptor ring bytes (4096 descs)
NSWQ = 4              # SWDGE queues (desc-gen parallelism)
IXSLAB = 8            # gather-idx chunks per ix DMA
PAD_SLOT = 1000.0
XSLAB = 4             # windows per xw DMA
LSLAB = 8             # windows per lr DMA

_cache = {}


def _preprocess_side(x, edge_index, batch):
    src = np.asarray(edge_index[0], np.int64)
    dst = np.asarray(edge_index[1], np.int64)
    batch = np.asarray(batch, np.int64)
    x = np.asarray(x, np.float32)

    deg = np.bincount(dst, minlength=N).astype(np.float64) + 1.0
    dinv = (1.0 / np.sqrt(deg)).astype(np.float32)

    sl = np.arange(N, dtype=np.int64)
    asrc = np.concatenate([src, sl])
    adst = np.concatenate([dst, sl])

    norm64 = dinv[asrc].astype(np.float64) * dinv[adst].astype(np.float64)
    t_g = np.bincount(batch[adst], weights=norm64, minlength=G)
    n_g = np.bincount(batch, minlength=G).astype(np.float64)

    w = dinv[asrc] * dinv[asrc] * dinv[adst]
    chat = np.bincount(asrc * G + batch[adst], weights=w.astype(np.float64),
                       minlength=N * G).reshape(N, G).astype(np.float32)
    xhat = dinv[:, None] * x

    outdeg = np.bincount(asrc, minlength=N)
    cum = np.cumsum(outdeg)
    targets = np.arange(1, NCORES) * (cum[-1] / NCORES)
    bounds = np.searchsorted(cum, targets)
    node_lo = np.concatenate([[0], bounds + 1]).astype(np.int64)
    node_hi = np.concatenate([bounds + 1, [N]]).astype(np.int64)

    order = np.argsort(asrc, kind="stable")
    asrc_s, adst_s = asrc[order], adst[order]
    core_e0 = np.searchsorted(asrc_s, node_lo)
    core_e1 = np.searchsorted(asrc_s, node_hi)

    nw = int(max(-(-(node_hi - node_lo) // 128)))
    return dict(chat=chat, xhat=xhat, t_g=t_g, n_g=n_g,
                node_lo=node_lo, node_hi=node_hi,
                asrc_s=asrc_s, adst_s=adst_s,
                core_e0=core_e0, core_e1=core_e1, nw=nw)


def _pack_core(prep, c, nw, nch):
    """Device inputs for core c, plus the exact host spill contribution."""
    lo, hi = int(prep["node_lo"][c]), int(prep["node_hi"][c])
    es, ee = int(prep["core_e0"][c]), int(prep["core_e1"][c])
    s2 = prep["asrc_s"][es:ee]
    d2 = prep["adst_s"][es:ee]
    xhat, chat = prep["xhat"], prep["chat"]

    nwc = -(-(hi - lo) // 128)
    nxs, nls = -(-nw // XSLAB), -(-nw // LSLAB)
    lrt = np.full((nls, 128, LSLAB, WCOLS), PAD_SLOT, np.float32)
    xwp = np.zeros((nxs, 128, XSLAB, DIN), np.float32)
    streams = np.zeros((NRES, nw * WB * 128), np.int16)
    spill_s, spill_d = [], []

    res = (d2 % NRES).astype(np.int64)
    d4 = (d2 // NRES).astype(np.int64)
    for w in range(nwc):
        base = lo + 128 * w
        a = np.searchsorted(s2, base)
        b = np.searchsorted(s2, base + 128)
        n_nodes = min(128, hi - base)
        xwp[w // XSLAB, :n_nodes, w % XSLAB, :] = xhat[base:base + n_nodes]
        sw_, rw_, dw_, dfull = s2[a:b], res[a:b], d4[a:b], d2[a:b]
        for r in range(NRES):
            m = rw_ == r
            ss, dd = sw_[m], dw_[m]
            if len(ss) > CAP:
                spill_s.append(sw_[m][CAP:])
                spill_d.append(dfull[m][CAP:])
                ss, dd = ss[:CAP], dd[:CAP]
            cnt = len(ss)
            slots = (ss - base).astype(np.float32)
            col = np.full(WB * 128, PAD_SLOT, np.float32)
            col[:cnt] = slots
            lrt[w // LSLAB, :, w % LSLAB, r * WB:(r + 1) * WB] = \
                col.reshape(WB, 128).T
            seg = np.zeros(WB * 128, np.int16)
            seg[:cnt] = dd.astype(np.int16)
            streams[r, w * WB * 128:(w + 1) * WB * 128] = seg

    C = CHUNK // 16
    nslab = -(-nch // IXSLAB)
    ix = np.zeros((NRES, nslab, 128, IXSLAB * C), np.int16)
    j = np.arange(CHUNK)
    for r in range(NRES):
        flat = np.zeros(nch * CHUNK, np.int16)
        flat[:nw * WB * 128] = streams[r]
        for q in range(nch):
            wrap = np.zeros((16, C), np.int16)
            wrap[j % 16, j // 16] = flat[q * CHUNK:(q + 1) * CHUNK]
            ix[r, q // IXSLAB, :, (q % IXSLAB) * C:(q % IXSLAB + 1) * C] = \
                np.tile(wrap, (8, 1))

    if spill_s:
        ss = np.concatenate(spill_s)
        dd = np.concatenate(spill_d)
        p_spill = chat[dd].astype(np.float64).T @ xhat[ss].astype(np.float64)
    else:
        p_spill = np.zeros((G, DIN), np.float64)
    return dict(lrt=lrt, xwp=xwp, ix=ix), p_spill


def _build_nc(nw, nch, reps=1):
    import concourse.bacc as bacc
    import concourse.mybir as mybir
    import concourse.tile as tile

    nc = bacc.Bacc("TRN2", target_bir_lowering=False, debug=False,
                   num_devices=NCORES, dynamic_dma_scratch_size=SCRATCH,
                   num_swdge_queues=NSWQ)
    f32, i16 = mybir.dt.float32, mybir.dt.int16

    nxs, nls = -(-nw // XSLAB), -(-nw // LSLAB)
    ch = [nc.dram_tensor(f"ch{s}", [N, G], f32, kind="ExternalInput")
          for s in range(2)]
    xwp = [nc.dram_tensor(f"xwp{s}", [nxs, 128, XSLAB, DIN], f32,
                          kind="ExternalInput") for s in range(2)]
    lrt = [nc.dram_tensor(f"lrt{s}", [nls, 128, LSLAB, WCOLS], f32,
                          kind="ExternalInput") for s in range(2)]
    nslab = -(-nch // IXSLAB)
    ixt = [nc.dram_tensor(f"ix{s}", [NRES, nslab, 128, IXSLAB * (CHUNK // 16)],
                          i16, kind="ExternalInput") for s in range(2)]
    iota = nc.dram_tensor("iota", [128, 128], f32, kind="ExternalInput")
    pout = [nc.dram_tensor(f"P{s}", [G, DIN], f32, kind="ExternalOutput")
            for s in range(2)]

    with tile.TileContext(nc) as tc:
        with tc.tile_pool(name="const", bufs=1) as cpool, \
             tc.tile_pool(name="ix", bufs=3) as ixpool, \
             tc.tile_pool(name="g0", bufs=3) as gp0, \
             tc.tile_pool(name="g1", bufs=3) as gp1, \
             tc.tile_pool(name="g2", bufs=3) as gp2, \
             tc.tile_pool(name="g3", bufs=3) as gp3, \
             tc.tile_pool(name="xw", bufs=3) as xpool, \
             tc.tile_pool(name="lr", bufs=2) as lpool, \
             tc.tile_pool(name="oh", bufs=2) as opool, \
             tc.tile_pool(name="sb", bufs=3) as spool, \
             tc.tile_pool(name="sw", bufs=2, space="PSUM") as swpool, \
             tc.tile_pool(name="pp", bufs=2, space="PSUM") as ppool:
            gpools = [gp0, gp1, gp2, gp3]
            it = cpool.tile([128, 128], f32)
            nc.sync.dma_start(out=it[:], in_=iota[:, :])
            for s in [s for _ in range(reps) for s in range(2)]:
                chv = ch[s][:, :].rearrange("(a b) g -> a b g", b=NRES)
                pacc = ppool.tile([G, DIN], f32)
                rings = [dict() for _ in range(NRES)]
                ixslabs = [None] * NRES
                issued = [0] * NRES
                for w in range(nw):
                    if w % XSLAB == 0:
                        xws = xpool.tile([128, XSLAB, DIN], f32, tag="xw")
                        nc.sync.dma_start(out=xws[:],
                                          in_=xwp[s][w // XSLAB, :, :, :])
                    if w % LSLAB == 0:
                        lrs = lpool.tile([128, LSLAB, WCOLS], f32, tag="lr")
                        nc.sync.dma_start(out=lrs[:],
                                          in_=lrt[s][w // LSLAB, :, :, :])
                    C = CHUNK // 16
                    for r in range(NRES):
                        while issued[r] * CBLK < min((w + 2) * WB, nw * WB):
                            q = issued[r]
                            if q % IXSLAB == 0:
                                ixslabs[r] = ixpool.tile(
                                    [128, IXSLAB * C], i16, tag=f"ix{r}",
                                    name=f"ixs{r}")
                                nc.sync.dma_start(
                                    out=ixslabs[r][:],
                                    in_=ixt[s][r, q // IXSLAB, :, :])
                            ixq = ixslabs[r][:, (q % IXSLAB) * C:
                                             (q % IXSLAB + 1) * C]
                            gt = gpools[r].tile([128, CBLK, G], f32,
                                                tag=f"g{r}")
                            nc.gpsimd.dma_gather(
                                gt[:], chv[:, r, :], ixq, CHUNK, CHUNK, G,
                                elem_step=NRES * G, queue_num=r % NSWQ)
                            rings[r][q] = gt
                            issued[r] += 1
                    oh = opool.tile([128, WCOLS, 128], f32, tag="oh")
                    lwb = lrs[:, w % LSLAB, :].rearrange(
                        "p (b o) -> p b o", o=1).to_broadcast([128, WCOLS, 128])
                    iob = it[:].rearrange("p (a c) -> p a c", a=1) \
                               .to_broadcast([128, WCOLS, 128])
                    nc.vector.tensor_tensor(out=oh[:], in0=iob, in1=lwb,
                                            op=mybir.AluOpType.is_equal)
                    sw = swpool.tile([128, G], f32)
                    for r in range(NRES):
                        for k in range(WB):
                            q, jj = divmod(w * WB + k, CBLK)
                            bi = r * WB + k
                            nc.tensor.matmul(
                                out=sw[:, :], lhsT=oh[:, bi, :],
                                rhs=rings[r][q][:, jj, :],
                                start=(bi == 0), stop=(bi == WCOLS - 1))
                    swb = spool.tile([128, G], f32, tag="swb")
                    nc.scalar.copy(out=swb[:], in_=sw[:, :])
                    nc.tensor.matmul(out=pacc[:, :], lhsT=swb[:, :],
                                     rhs=xws[:, w % XSLAB, :],
                                     start=(w == 0), stop=(w == nw - 1))
                pf = spool.tile([G, DIN], f32, tag="pf")
                nc.scalar.copy(out=pf[:], in_=pacc[:, :])
                nc.sync.dma_start(out=pout[s][:, :], in_=pf[:])
    nc.compile()
    return nc


def kernel(x0, x1, edge_index0, edge_index1, batch0, batch1,
           W1, b1, W2, b2, Wc1, bc1, Wc2, bc2):
    from concourse import bass_utils

    prep0 = _preprocess_side(x0, edge_index0, batch0)
    prep1 = _preprocess_side(x1, edge_index1, batch1)

    nw = max(prep0["nw"], prep1["nw"])
    nch = -(-(nw * WB * 128) // CHUNK)

    key = (nw, nch)
    if key not in _cache:
        _cache[key] = _build_nc(nw, nch)
    nc = _cache[key]

    iota = np.tile(np.arange(128, dtype=np.float32)[None, :], (128, 1))
    in_maps = []
    spills = [np.zeros((G, DIN), np.float64) for _ in range(2)]
    for c in range(NCORES):
        m = dict(iota=iota)
        for s, prep in ((0, prep0), (1, prep1)):
            pk, p_spill = _pack_core(prep, c, nw, nch)
            spills[s] += p_spill
            m[f"ch{s}"] = prep["chat"]
            m[f"xwp{s}"] = pk["xwp"]
            m[f"lrt{s}"] = pk["lrt"]
            m[f"ix{s}"] = pk["ix"]
        in_maps.append(m)

    res = bass_utils.run_bass_kernel_spmd(nc, in_maps, core_ids=list(range(NCORES)))
    kernel.last_results = res
    kernel.last_nc = nc
    kernel.last_in_maps = in_maps

    P0 = spills[0].copy()
    P1 = spills[1].copy()
    for c in range(NCORES):
        P0 += res.results[c]["P0"]
        P1 += res.results[c]["P1"]

    # host finish: tiny pooled + compare MLP (4 MFLOP)
    W1 = np.asarray(W1, np.float32); W2 = np.asarray(W2, np.float32)
    Wp = W1 @ W2
    bp1 = np.asarray(b1, np.float32) @ W2

    def pooled(P, t, n):
        out = (P.astype(np.float32) @ Wp + t[:, None].astype(np.float32) * bp1[None, :]
               + n[:, None].astype(np.float32) * np.asarray(b2, np.float32)[None, :])
        return out / np.maximum(n, 1.0)[:, None].astype(np.float32)

    cfeat = np.concatenate([pooled(P0, prep0["t_g"], prep0["n_g"]),
                            pooled(P1, prep1["t_g"], prep1["n_g"])], axis=1)
    h = 1.0 / (1.0 + np.exp(-(cfeat @ np.asarray(Wc1, np.float32)
                              + np.asarray(bc1, np.float32))))
    prob = 1.0 / (1.0 + np.exp(-(h @ np.asarray(Wc2, np.float32)
                                 + np.asarray(bc2, np.float32))))
    return prob[:, 0].astype(np.float32)


# revision 4
# speedup vs baseline: 1.0196x; 1.0196x over previous
"""Trainium2 Bass kernel for nn_CircuitRankNet (2-layer GCN siamese + mean-pool + MLP).

Algebraic collapse (no nonlinearity between the GCN layers): with
M = D^-1/2 (A+I) D^-1/2 and B = onehot(batch), the pooled embeddings need only
    P[g,d] = sum_over_aug_edges Chat[dst_e, g] * Xhat[src_e, d]
where Chat[i,:] = dinv_i^2 * sum_{e: src=i} dinv_dst * onehot64(batch[dst])
and Xhat[j,:] = dinv_j * X[j,:] (both computed on host from indices/degrees).

Device evaluation (v3, src-sharded):
- nodes split into 8 contiguous src-ranges (aug-outdegree balanced), one per
  NeuronCore; windows = fixed 128-node spans of the local range.
- per window, its incident edges (by src) are bucketed by residue
  r = dst % 4 and padded into exactly WB blocks of 128 edges per residue.
  Residue views chat[(25000,4,64)][:, r, :] keep gather indices (dst//4)
  int16-safe, with uniform density across windows.
- per residue, the window-major block stream is gathered from HBM in
  CHUNK-index dma_gather calls (8-way parallel Q7 descriptor generation)
  into an SBUF ring.
- per window: one-hot of the src slot (iota compare), per-block matmuls
  accumulate S_w[slot, g] in PSUM, then P += S_w^T @ Xhat[window rows]
  (second matmul, PSUM-accumulated across all windows).
- rare (window, residue) buckets over capacity spill to an exact host-side
  outer-product fixup; per-graph P partials are summed on the host and fed
  to the tiny compare MLP.
"""
import numpy as np

NCORES = 8
N = 100000
E = 1600000
G = 64
DIN = 128
NRES = 4
NQ = N // NRES        # 25000 rows per residue view
WB = 5                # blocks (of 128 edges) per residue per window
WCOLS = NRES * WB     # 20 one-hot columns per window
CAP = WB * 128        # 640-edge capacity per (window, residue)
CHUNK = 1024          # idxs per dma_gather call
CBLK = CHUNK // 128   # blocks per chunk
SCRATCH = 65536       # SWDGE descriptor ring bytes (4096 descs)
NSWQ = 4              # SWDGE queues (desc-gen parallelism)
IXSLAB = 8            # gather-idx chunks per ix DMA
PAD_SLOT = 1000.0
XSLAB = 4             # windows per xw DMA
LSLAB = 8             # windows per lr DMA

_cache = {}


def _preprocess_side(x, edge_index, batch):
    src = np.asarray(edge_index[0], np.int64)
    dst = np.asarray(edge_index[1], np.int64)
    batch = np.asarray(batch, np.int64)
    x = np.asarray(x, np.float32)

    deg = np.bincount(dst, minlength=N).astype(np.float64) + 1.0
    dinv = (1.0 / np.sqrt(deg)).astype(np.float32)

    sl = np.arange(N, dtype=np.int64)
    asrc = np.concatenate([src, sl])
    adst = np.concatenate([dst, sl])

    norm64 = dinv[asrc].astype(np.float64) * dinv[adst].astype(np.float64)
    t_g = np.bincount(batch[adst], weights=norm64, minlength=G)
    n_g = np.bincount(batch, minlength=G).astype(np.float64)

    w = dinv[asrc] * dinv[asrc] * dinv[adst]
    chat = np.bincount(asrc * G + batch[adst], weights=w.astype(np.float64),
                       minlength=N * G).reshape(N, G).astype(np.float32)
    xhat = dinv[:, None] * x

    outdeg = np.bincount(asrc, minlength=N)
    cum = np.cumsum(outdeg)
    targets = np.arange(1, NCORES) * (cum[-1] / NCORES)
    bounds = np.searchsorted(cum, targets)
    node_lo = np.concatenate([[0], bounds + 1]).astype(np.int64)
    node_hi = np.concatenate([bounds + 1, [N]]).astype(np.int64)

    order = np.argsort(asrc, kind="stable")
    asrc_s, adst_s = asrc[order], adst[order]
    core_e0 = np.searchsorted(asrc_s, node_lo)
    core_e1 = np.searchsorted(asrc_s, node_hi)

    nw = int(max(-(-(node_hi - node_lo) // 128)))
    return dict(chat=chat, xhat=xhat, t_g=t_g, n_g=n_g,
                node_lo=node_lo, node_hi=node_hi,
                asrc_s=asrc_s, adst_s=adst_s,
                core_e0=core_e0, core_e1=core_e1, nw=nw)


def _pack_core(prep, c, nw, nch):
    """Device inputs for core c, plus the exact host spill contribution."""
    lo, hi = int(prep["node_lo"][c]), int(prep["node_hi"][c])
    es, ee = int(prep["core_e0"][c]), int(prep["core_e1"][c])
    s2 = prep["asrc_s"][es:ee]
    d2 = prep["adst_s"][es:ee]
    xhat, chat = prep["xhat"], prep["chat"]

    nwc = -(-(hi - lo) // 128)
    nxs, nls = -(-nw // XSLAB), -(-nw // LSLAB)
    lrt = np.full((nls, 128, LSLAB, WCOLS), PAD_SLOT, np.float32)  # ->bf16 at end
    xwp = np.zeros((nxs, 128, XSLAB, DIN), np.float32)
    streams = np.zeros((NRES, nw * WB * 128), np.int16)
    spill_s, spill_d = [], []

    res = (d2 % NRES).astype(np.int64)
    d4 = (d2 // NRES).astype(np.int64)
    for w in range(nwc):
        base = lo + 128 * w
        a = np.searchsorted(s2, base)
        b = np.searchsorted(s2, base + 128)
        n_nodes = min(128, hi - base)
        xwp[w // XSLAB, :n_nodes, w % XSLAB, :] = xhat[base:base + n_nodes]
        sw_, rw_, dw_, dfull = s2[a:b], res[a:b], d4[a:b], d2[a:b]
        for r in range(NRES):
            m = rw_ == r
            ss, dd = sw_[m], dw_[m]
            if len(ss) > CAP:
                spill_s.append(sw_[m][CAP:])
                spill_d.append(dfull[m][CAP:])
                ss, dd = ss[:CAP], dd[:CAP]
            cnt = len(ss)
            slots = (ss - base).astype(np.float32)
            col = np.full(WB * 128, PAD_SLOT, np.float32)
            col[:cnt] = slots
            lrt[w // LSLAB, :, w % LSLAB, r * WB:(r + 1) * WB] = \
                col.reshape(WB, 128).T
            seg = np.zeros(WB * 128, np.int16)
            seg[:cnt] = dd.astype(np.int16)
            streams[r, w * WB * 128:(w + 1) * WB * 128] = seg

    C = CHUNK // 16
    nslab = -(-nch // IXSLAB)
    ix = np.zeros((NRES, nslab, 128, IXSLAB * C), np.int16)
    j = np.arange(CHUNK)
    for r in range(NRES):
        flat = np.zeros(nch * CHUNK, np.int16)
        flat[:nw * WB * 128] = streams[r]
        for q in range(nch):
            wrap = np.zeros((16, C), np.int16)
            wrap[j % 16, j // 16] = flat[q * CHUNK:(q + 1) * CHUNK]
            ix[r, q // IXSLAB, :, (q % IXSLAB) * C:(q % IXSLAB + 1) * C] = \
                np.tile(wrap, (8, 1))

    from ml_dtypes import bfloat16 as _bf16
    lrt = lrt.astype(_bf16)
    if spill_s:
        ss = np.concatenate(spill_s)
        dd = np.concatenate(spill_d)
        p_spill = chat[dd].astype(np.float64).T @ xhat[ss].astype(np.float64)
    else:
        p_spill = np.zeros((G, DIN), np.float64)
    return dict(lrt=lrt, xwp=xwp, ix=ix), p_spill


def _build_nc(nw, nch, reps=1):
    import concourse.bacc as bacc
    import concourse.mybir as mybir
    import concourse.tile as tile

    nc = bacc.Bacc("TRN2", target_bir_lowering=False, debug=False,
                   num_devices=NCORES, dynamic_dma_scratch_size=SCRATCH,
                   num_swdge_queues=NSWQ)
    f32, i16 = mybir.dt.float32, mybir.dt.int16
    bf16 = mybir.dt.bfloat16

    nxs, nls = -(-nw // XSLAB), -(-nw // LSLAB)
    ch = [nc.dram_tensor(f"ch{s}", [N, G], f32, kind="ExternalInput")
          for s in range(2)]
    xwp = [nc.dram_tensor(f"xwp{s}", [nxs, 128, XSLAB, DIN], f32,
                          kind="ExternalInput") for s in range(2)]
    lrt = [nc.dram_tensor(f"lrt{s}", [nls, 128, LSLAB, WCOLS], bf16,
                          kind="ExternalInput") for s in range(2)]
    nslab = -(-nch // IXSLAB)
    ixt = [nc.dram_tensor(f"ix{s}", [NRES, nslab, 128, IXSLAB * (CHUNK // 16)],
                          i16, kind="ExternalInput") for s in range(2)]
    iota = nc.dram_tensor("iota", [128, 128], bf16, kind="ExternalInput")
    pout = [nc.dram_tensor(f"P{s}", [G, DIN], f32, kind="ExternalOutput")
            for s in range(2)]

    with tile.TileContext(nc) as tc:
        with tc.tile_pool(name="const", bufs=1) as cpool, \
             tc.tile_pool(name="ix", bufs=3) as ixpool, \
             tc.tile_pool(name="g0", bufs=3) as gp0, \
             tc.tile_pool(name="g1", bufs=3) as gp1, \
             tc.tile_pool(name="g2", bufs=3) as gp2, \
             tc.tile_pool(name="g3", bufs=3) as gp3, \
             tc.tile_pool(name="gb0", bufs=3) as gb0, \
             tc.tile_pool(name="gb1", bufs=3) as gb1, \
             tc.tile_pool(name="gb2", bufs=3) as gb2, \
             tc.tile_pool(name="gb3", bufs=3) as gb3, \
             tc.tile_pool(name="xw", bufs=3) as xpool, \
             tc.tile_pool(name="lr", bufs=2) as lpool, \
             tc.tile_pool(name="oh", bufs=2) as opool, \
             tc.tile_pool(name="sb", bufs=3) as spool, \
             tc.tile_pool(name="sw", bufs=2, space="PSUM") as swpool, \
             tc.tile_pool(name="pp", bufs=2, space="PSUM") as ppool:
            gpools = [gp0, gp1, gp2, gp3]
            gbpools = [gb0, gb1, gb2, gb3]
            it = cpool.tile([128, 128], bf16)
            nc.sync.dma_start(out=it[:], in_=iota[:, :])
            for s in [s for _ in range(reps) for s in range(2)]:
                chv = ch[s][:, :].rearrange("(a b) g -> a b g", b=NRES)
                pacc = ppool.tile([G, DIN], f32)
                rings = [dict() for _ in range(NRES)]
                ixslabs = [None] * NRES
                issued = [0] * NRES
                for w in range(nw):
                    if w % XSLAB == 0:
                        xws = xpool.tile([128, XSLAB, DIN], f32, tag="xw")
                        nc.sync.dma_start(out=xws[:],
                                          in_=xwp[s][w // XSLAB, :, :, :])
                    if w % LSLAB == 0:
                        lrs = lpool.tile([128, LSLAB, WCOLS], bf16, tag="lr")
                        nc.sync.dma_start(out=lrs[:],
                                          in_=lrt[s][w // LSLAB, :, :, :])
                    C = CHUNK // 16
                    for r in range(NRES):
                        while issued[r] * CBLK < min((w + 2) * WB, nw * WB):
                            q = issued[r]
                            if q % IXSLAB == 0:
                                ixslabs[r] = ixpool.tile(
                                    [128, IXSLAB * C], i16, tag=f"ix{r}",
                                    name=f"ixs{r}")
                                nc.sync.dma_start(
                                    out=ixslabs[r][:],
                                    in_=ixt[s][r, q // IXSLAB, :, :])
                            ixq = ixslabs[r][:, (q % IXSLAB) * C:
                                             (q % IXSLAB + 1) * C]
                            gt = gpools[r].tile([128, CBLK, G], f32,
                                                tag=f"g{r}")
                            nc.gpsimd.dma_gather(
                                gt[:], chv[:, r, :], ixq, CHUNK, CHUNK, G,
                                elem_step=NRES * G, queue_num=r % NSWQ)
                            gtb = gbpools[r].tile([128, CBLK, G], bf16,
                                                  name=f"gtb{r}",
                                                  tag=f"gb{r}")
                            nc.scalar.copy(out=gtb[:], in_=gt[:])
                            rings[r][q] = gtb
                            issued[r] += 1
                    oh = opool.tile([128, WCOLS, 128], bf16, tag="oh")
                    lwb = lrs[:, w % LSLAB, :].rearrange(
                        "p (b o) -> p b o", o=1).to_broadcast([128, WCOLS, 128])
                    iob = it[:].rearrange("p (a c) -> p a c", a=1) \
                               .to_broadcast([128, WCOLS, 128])
                    nc.vector.tensor_tensor(out=oh[:], in0=iob, in1=lwb,
                                            op=mybir.AluOpType.is_equal)
                    sw = swpool.tile([128, G], f32)
                    for r in range(NRES):
                        for k in range(WB):
                            q, jj = divmod(w * WB + k, CBLK)
                            bi = r * WB + k
                            nc.tensor.matmul(
                                out=sw[:, :], lhsT=oh[:, bi, :],
                                rhs=rings[r][q][:, jj, :],
                                start=(bi == 0), stop=(bi == WCOLS - 1))
                    swb = spool.tile([128, G], f32, tag="swb")
                    nc.scalar.copy(out=swb[:], in_=sw[:, :])
                    nc.tensor.matmul(out=pacc[:, :], lhsT=swb[:, :],
                                     rhs=xws[:, w % XSLAB, :],
                                     start=(w == 0), stop=(w == nw - 1))
                pf = spool.tile([G, DIN], f32, tag="pf")
                nc.scalar.copy(out=pf[:], in_=pacc[:, :])
                nc.sync.dma_start(out=pout[s][:, :], in_=pf[:])
    nc.compile()
    return nc


def kernel(x0, x1, edge_index0, edge_index1, batch0, batch1,
           W1, b1, W2, b2, Wc1, bc1, Wc2, bc2):
    from concourse import bass_utils

    prep0 = _preprocess_side(x0, edge_index0, batch0)
    prep1 = _preprocess_side(x1, edge_index1, batch1)

    nw = max(prep0["nw"], prep1["nw"])
    nch = -(-(nw * WB * 128) // CHUNK)

    key = (nw, nch)
    if key not in _cache:
        _cache[key] = _build_nc(nw, nch)
    nc = _cache[key]

    from ml_dtypes import bfloat16 as _bf16
    iota = np.tile(np.arange(128, dtype=np.float32)[None, :],
                   (128, 1)).astype(_bf16)
    in_maps = []
    spills = [np.zeros((G, DIN), np.float64) for _ in range(2)]
    for c in range(NCORES):
        m = dict(iota=iota)
        for s, prep in ((0, prep0), (1, prep1)):
            pk, p_spill = _pack_core(prep, c, nw, nch)
            spills[s] += p_spill
            m[f"ch{s}"] = prep["chat"]
            m[f"xwp{s}"] = pk["xwp"]
            m[f"lrt{s}"] = pk["lrt"]
            m[f"ix{s}"] = pk["ix"]
        in_maps.append(m)

    res = bass_utils.run_bass_kernel_spmd(nc, in_maps, core_ids=list(range(NCORES)))
    kernel.last_results = res
    kernel.last_nc = nc
    kernel.last_in_maps = in_maps

    P0 = spills[0].copy()
    P1 = spills[1].copy()
    for c in range(NCORES):
        P0 += res.results[c]["P0"]
        P1 += res.results[c]["P1"]

    # host finish: tiny pooled + compare MLP (4 MFLOP)
    W1 = np.asarray(W1, np.float32); W2 = np.asarray(W2, np.float32)
    Wp = W1 @ W2
    bp1 = np.asarray(b1, np.float32) @ W2

    def pooled(P, t, n):
        out = (P.astype(np.float32) @ Wp + t[:, None].astype(np.float32) * bp1[None, :]
               + n[:, None].astype(np.float32) * np.asarray(b2, np.float32)[None, :])
        return out / np.maximum(n, 1.0)[:, None].astype(np.float32)

    cfeat = np.concatenate([pooled(P0, prep0["t_g"], prep0["n_g"]),
                            pooled(P1, prep1["t_g"], prep1["n_g"])], axis=1)
    h = 1.0 / (1.0 + np.exp(-(cfeat @ np.asarray(Wc1, np.float32)
                              + np.asarray(bc1, np.float32))))
    prob = 1.0 / (1.0 + np.exp(-(h @ np.asarray(Wc2, np.float32)
                                 + np.asarray(bc2, np.float32))))
    return prob[:, 0].astype(np.float32)


# revision 5
# speedup vs baseline: 1.0913x; 1.0703x over previous
"""Trainium2 Bass kernel for nn_CircuitRankNet (2-layer GCN siamese + mean-pool + MLP).

Algebraic collapse (no nonlinearity between the GCN layers): with
M = D^-1/2 (A+I) D^-1/2 and B = onehot(batch), the pooled embeddings need only
    P[g,d] = sum_over_aug_edges Chat[dst_e, g] * Xhat[src_e, d]
where Chat[i,:] = dinv_i^2 * sum_{e: src=i} dinv_dst * onehot64(batch[dst])
and Xhat[j,:] = dinv_j * X[j,:] (both computed on host from indices/degrees).

Device evaluation (v3, src-sharded):
- nodes split into 8 contiguous src-ranges (aug-outdegree balanced), one per
  NeuronCore; windows = fixed 128-node spans of the local range.
- per window, its incident edges (by src) are bucketed by residue
  r = dst % 4 and padded into exactly WB blocks of 128 edges per residue.
  Residue views chat[(25000,4,64)][:, r, :] keep gather indices (dst//4)
  int16-safe, with uniform density across windows.
- per residue, the window-major block stream is gathered from HBM in
  CHUNK-index dma_gather calls (8-way parallel Q7 descriptor generation)
  into an SBUF ring.
- per window: one-hot of the src slot (iota compare), per-block matmuls
  accumulate S_w[slot, g] in PSUM, then P += S_w^T @ Xhat[window rows]
  (second matmul, PSUM-accumulated across all windows).
- rare (window, residue) buckets over capacity spill to an exact host-side
  outer-product fixup; per-graph P partials are summed on the host and fed
  to the tiny compare MLP.
"""
import numpy as np

NCORES = 8
N = 100000
E = 1600000
G = 64
DIN = 128
NRES = 4
NQ = N // NRES        # 25000 rows per residue view
WB = 5                # blocks (of 128 edges) per residue per window
WCOLS = NRES * WB     # 20 one-hot columns per window
CAP = WB * 128        # 640-edge capacity per (window, residue)
CHUNK = 1024          # idxs per dma_gather call (ucode limit < 1536)
CBLK = CHUNK // 128   # blocks per chunk
SCRATCH = 65536       # SWDGE descriptor ring bytes (4096 descs)
NSWQ = 4              # SWDGE queues (desc-gen parallelism)
IXSLAB = 8            # gather-idx chunks per ix DMA
PAD_SLOT = 1000.0
XSLAB = 4             # windows per xw DMA
LSLAB = 8             # windows per lr DMA

_cache = {}


def _preprocess_side(x, edge_index, batch):
    src = np.asarray(edge_index[0], np.int64)
    dst = np.asarray(edge_index[1], np.int64)
    batch = np.asarray(batch, np.int64)
    x = np.asarray(x, np.float32)

    deg = np.bincount(dst, minlength=N).astype(np.float64) + 1.0
    dinv = (1.0 / np.sqrt(deg)).astype(np.float32)

    sl = np.arange(N, dtype=np.int64)
    asrc = np.concatenate([src, sl])
    adst = np.concatenate([dst, sl])

    norm64 = dinv[asrc].astype(np.float64) * dinv[adst].astype(np.float64)
    t_g = np.bincount(batch[adst], weights=norm64, minlength=G)
    n_g = np.bincount(batch, minlength=G).astype(np.float64)

    w = dinv[asrc] * dinv[asrc] * dinv[adst]
    chat = np.bincount(asrc * G + batch[adst], weights=w.astype(np.float64),
                       minlength=N * G).reshape(N, G).astype(np.float32)
    xhat = dinv[:, None] * x

    outdeg = np.bincount(asrc, minlength=N)
    cum = np.cumsum(outdeg)
    targets = np.arange(1, NCORES) * (cum[-1] / NCORES)
    bounds = np.searchsorted(cum, targets)
    node_lo = np.concatenate([[0], bounds + 1]).astype(np.int64)
    node_hi = np.concatenate([bounds + 1, [N]]).astype(np.int64)

    order = np.argsort(asrc, kind="stable")
    asrc_s, adst_s = asrc[order], adst[order]
    core_e0 = np.searchsorted(asrc_s, node_lo)
    core_e1 = np.searchsorted(asrc_s, node_hi)

    nw = int(max(-(-(node_hi - node_lo) // 128)))
    return dict(chat=chat, xhat=xhat, t_g=t_g, n_g=n_g,
                node_lo=node_lo, node_hi=node_hi,
                asrc_s=asrc_s, adst_s=adst_s,
                core_e0=core_e0, core_e1=core_e1, nw=nw)


def _pack_core(prep, c, nw, nch):
    """Device inputs for core c, plus the exact host spill contribution."""
    lo, hi = int(prep["node_lo"][c]), int(prep["node_hi"][c])
    es, ee = int(prep["core_e0"][c]), int(prep["core_e1"][c])
    s2 = prep["asrc_s"][es:ee]
    d2 = prep["adst_s"][es:ee]
    xhat, chat = prep["xhat"], prep["chat"]

    nwc = -(-(hi - lo) // 128)
    nxs, nls = -(-nw // XSLAB), -(-nw // LSLAB)
    lrt = np.full((nls, 128, LSLAB, WCOLS), PAD_SLOT, np.float32)  # ->bf16 at end
    xwp = np.zeros((nxs, 128, XSLAB, DIN), np.float32)
    streams = np.zeros((NRES, nw * WB * 128), np.int16)
    spill_s, spill_d = [], []

    res = (d2 % NRES).astype(np.int64)
    d4 = (d2 // NRES).astype(np.int64)
    for w in range(nwc):
        base = lo + 128 * w
        a = np.searchsorted(s2, base)
        b = np.searchsorted(s2, base + 128)
        n_nodes = min(128, hi - base)
        xwp[w // XSLAB, :n_nodes, w % XSLAB, :] = xhat[base:base + n_nodes]
        sw_, rw_, dw_, dfull = s2[a:b], res[a:b], d4[a:b], d2[a:b]
        for r in range(NRES):
            m = rw_ == r
            ss, dd = sw_[m], dw_[m]
            if len(ss) > CAP:
                spill_s.append(sw_[m][CAP:])
                spill_d.append(dfull[m][CAP:])
                ss, dd = ss[:CAP], dd[:CAP]
            cnt = len(ss)
            slots = (ss - base).astype(np.float32)
            col = np.full(WB * 128, PAD_SLOT, np.float32)
            col[:cnt] = slots
            lrt[w // LSLAB, :, w % LSLAB, r * WB:(r + 1) * WB] = \
                col.reshape(WB, 128).T
            seg = np.zeros(WB * 128, np.int16)
            seg[:cnt] = dd.astype(np.int16)
            streams[r, w * WB * 128:(w + 1) * WB * 128] = seg

    C = CHUNK // 16
    nslab = -(-nch // IXSLAB)
    ix = np.zeros((NRES, nslab, 128, IXSLAB * C), np.int16)
    j = np.arange(CHUNK)
    for r in range(NRES):
        flat = np.zeros(nch * CHUNK, np.int16)
        flat[:nw * WB * 128] = streams[r]
        for q in range(nch):
            wrap = np.zeros((16, C), np.int16)
            wrap[j % 16, j // 16] = flat[q * CHUNK:(q + 1) * CHUNK]
            ix[r, q // IXSLAB, :, (q % IXSLAB) * C:(q % IXSLAB + 1) * C] = \
                np.tile(wrap, (8, 1))

    from ml_dtypes import bfloat16 as _bf16
    lrt = lrt.astype(_bf16)
    xwp = xwp.astype(_bf16)
    if spill_s:
        ss = np.concatenate(spill_s)
        dd = np.concatenate(spill_d)
        p_spill = chat[dd].astype(np.float64).T @ xhat[ss].astype(np.float64)
    else:
        p_spill = np.zeros((G, DIN), np.float64)
    return dict(lrt=lrt, xwp=xwp, ix=ix), p_spill


def _build_nc(nw, nch, reps=1):
    import concourse.bacc as bacc
    import concourse.mybir as mybir
    import concourse.tile as tile

    nc = bacc.Bacc("TRN2", target_bir_lowering=False, debug=False,
                   num_devices=NCORES, dynamic_dma_scratch_size=SCRATCH,
                   num_swdge_queues=NSWQ)
    f32, i16 = mybir.dt.float32, mybir.dt.int16
    bf16 = mybir.dt.bfloat16

    nxs, nls = -(-nw // XSLAB), -(-nw // LSLAB)
    ch = [nc.dram_tensor(f"ch{s}", [N, G], f32, kind="ExternalInput")
          for s in range(2)]
    xwp = [nc.dram_tensor(f"xwp{s}", [nxs, 128, XSLAB, DIN], bf16,
                          kind="ExternalInput") for s in range(2)]
    lrt = [nc.dram_tensor(f"lrt{s}", [nls, 128, LSLAB, WCOLS], bf16,
                          kind="ExternalInput") for s in range(2)]
    nslab = -(-nch // IXSLAB)
    ixt = [nc.dram_tensor(f"ix{s}", [NRES, nslab, 128, IXSLAB * (CHUNK // 16)],
                          i16, kind="ExternalInput") for s in range(2)]
    iota = nc.dram_tensor("iota", [128, 128], bf16, kind="ExternalInput")
    pout = [nc.dram_tensor(f"P{s}", [G, DIN], f32, kind="ExternalOutput")
            for s in range(2)]

    with tile.TileContext(nc) as tc:
        with tc.tile_pool(name="const", bufs=1) as cpool, \
             tc.tile_pool(name="ix", bufs=3) as ixpool, \
             tc.tile_pool(name="g0", bufs=4) as gp0, \
             tc.tile_pool(name="g1", bufs=4) as gp1, \
             tc.tile_pool(name="g2", bufs=4) as gp2, \
             tc.tile_pool(name="g3", bufs=4) as gp3, \
             tc.tile_pool(name="gb0", bufs=4) as gb0, \
             tc.tile_pool(name="gb1", bufs=4) as gb1, \
             tc.tile_pool(name="gb2", bufs=4) as gb2, \
             tc.tile_pool(name="gb3", bufs=4) as gb3, \
             tc.tile_pool(name="xw", bufs=3) as xpool, \
             tc.tile_pool(name="lr", bufs=2) as lpool, \
             tc.tile_pool(name="oh", bufs=2) as opool, \
             tc.tile_pool(name="sb", bufs=3) as spool, \
             tc.tile_pool(name="sw", bufs=2, space="PSUM") as swpool, \
             tc.tile_pool(name="pp", bufs=2, space="PSUM") as ppool:
            gpools = [gp0, gp1, gp2, gp3]
            gbpools = [gb0, gb1, gb2, gb3]
            it = cpool.tile([128, 128], bf16)
            nc.sync.dma_start(out=it[:], in_=iota[:, :])
            for s in [s for _ in range(reps) for s in range(2)]:
                chv = ch[s][:, :].rearrange("(a b) g -> a b g", b=NRES)
                pacc = ppool.tile([G, DIN], f32)
                rings = [dict() for _ in range(NRES)]
                ixslabs = [None] * NRES
                issued = [0] * NRES
                for w in range(nw):
                    if w % XSLAB == 0:
                        xws = xpool.tile([128, XSLAB, DIN], bf16, tag="xw")
                        nc.sync.dma_start(out=xws[:],
                                          in_=xwp[s][w // XSLAB, :, :, :])
                    if w % LSLAB == 0:
                        lrs = lpool.tile([128, LSLAB, WCOLS], bf16, tag="lr")
                        nc.sync.dma_start(out=lrs[:],
                                          in_=lrt[s][w // LSLAB, :, :, :])
                    C = CHUNK // 16
                    for r in range(NRES):
                        while issued[r] * CBLK < min((w + 2) * WB, nw * WB):
                            q = issued[r]
                            if q % IXSLAB == 0:
                                ixslabs[r] = ixpool.tile(
                                    [128, IXSLAB * C], i16, tag=f"ix{r}",
                                    name=f"ixs{r}")
                                nc.sync.dma_start(
                                    out=ixslabs[r][:],
                                    in_=ixt[s][r, q // IXSLAB, :, :])
                            ixq = ixslabs[r][:, (q % IXSLAB) * C:
                                             (q % IXSLAB + 1) * C]
                            gt = gpools[r].tile([128, CBLK, G], f32,
                                                tag=f"g{r}")
                            nc.gpsimd.dma_gather(
                                gt[:], chv[:, r, :], ixq, CHUNK, CHUNK, G,
                                elem_step=NRES * G, queue_num=r % NSWQ)
                            gtb = gbpools[r].tile([128, CBLK, G], bf16,
                                                  name=f"gtb{r}",
                                                  tag=f"gb{r}")
                            nc.scalar.copy(out=gtb[:], in_=gt[:])
                            rings[r][q] = gtb
                            issued[r] += 1
                    oh = opool.tile([128, WCOLS, 128], bf16, tag="oh")
                    lwb = lrs[:, w % LSLAB, :].rearrange(
                        "p (b o) -> p b o", o=1).to_broadcast([128, WCOLS, 128])
                    iob = it[:].rearrange("p (a c) -> p a c", a=1) \
                               .to_broadcast([128, WCOLS, 128])
                    nc.vector.tensor_tensor(out=oh[:], in0=iob, in1=lwb,
                                            op=mybir.AluOpType.is_equal)
                    sw = swpool.tile([128, G], f32)
                    for r in range(NRES):
                        for k in range(WB):
                            q, jj = divmod(w * WB + k, CBLK)
                            bi = r * WB + k
                            nc.tensor.matmul(
                                out=sw[:, :], lhsT=oh[:, bi, :],
                                rhs=rings[r][q][:, jj, :],
                                start=(bi == 0), stop=(bi == WCOLS - 1))
                    swb = spool.tile([128, G], bf16, tag="swb")
                    nc.scalar.copy(out=swb[:], in_=sw[:, :])
                    nc.tensor.matmul(out=pacc[:, :], lhsT=swb[:, :],
                                     rhs=xws[:, w % XSLAB, :],
                                     start=(w == 0), stop=(w == nw - 1))
                pf = spool.tile([G, DIN], f32, tag="pf")
                nc.scalar.copy(out=pf[:], in_=pacc[:, :])
                nc.sync.dma_start(out=pout[s][:, :], in_=pf[:])
    nc.compile()
    return nc


def kernel(x0, x1, edge_index0, edge_index1, batch0, batch1,
           W1, b1, W2, b2, Wc1, bc1, Wc2, bc2):
    from concourse import bass_utils

    prep0 = _preprocess_side(x0, edge_index0, batch0)
    prep1 = _preprocess_side(x1, edge_index1, batch1)

    nw = max(prep0["nw"], prep1["nw"])
    nch = -(-(nw * WB * 128) // CHUNK)

    key = (nw, nch)
    if key not in _cache:
        _cache[key] = _build_nc(nw, nch)
    nc = _cache[key]

    from ml_dtypes import bfloat16 as _bf16
    iota = np.tile(np.arange(128, dtype=np.float32)[None, :],
                   (128, 1)).astype(_bf16)
    in_maps = []
    spills = [np.zeros((G, DIN), np.float64) for _ in range(2)]
    for c in range(NCORES):
        m = dict(iota=iota)
        for s, prep in ((0, prep0), (1, prep1)):
            pk, p_spill = _pack_core(prep, c, nw, nch)
            spills[s] += p_spill
            m[f"ch{s}"] = prep["chat"]
            m[f"xwp{s}"] = pk["xwp"]
            m[f"lrt{s}"] = pk["lrt"]
            m[f"ix{s}"] = pk["ix"]
        in_maps.append(m)

    res = bass_utils.run_bass_kernel_spmd(nc, in_maps, core_ids=list(range(NCORES)))
    kernel.last_results = res
    kernel.last_nc = nc
    kernel.last_in_maps = in_maps

    P0 = spills[0].copy()
    P1 = spills[1].copy()
    for c in range(NCORES):
        P0 += res.results[c]["P0"]
        P1 += res.results[c]["P1"]

    # host finish: tiny pooled + compare MLP (4 MFLOP)
    W1 = np.asarray(W1, np.float32); W2 = np.asarray(W2, np.float32)
    Wp = W1 @ W2
    bp1 = np.asarray(b1, np.float32) @ W2

    def pooled(P, t, n):
        out = (P.astype(np.float32) @ Wp + t[:, None].astype(np.float32) * bp1[None, :]
               + n[:, None].astype(np.float32) * np.asarray(b2, np.float32)[None, :])
        return out / np.maximum(n, 1.0)[:, None].astype(np.float32)

    cfeat = np.concatenate([pooled(P0, prep0["t_g"], prep0["n_g"]),
                            pooled(P1, prep1["t_g"], prep1["n_g"])], axis=1)
    h = 1.0 / (1.0 + np.exp(-(cfeat @ np.asarray(Wc1, np.float32)
                              + np.asarray(bc1, np.float32))))
    prob = 1.0 / (1.0 + np.exp(-(h @ np.asarray(Wc2, np.float32)
                                 + np.asarray(bc2, np.float32))))
    return prob[:, 0].astype(np.float32)
